# revision 127
# baseline (speedup 1.0000x reference)
"""Trainium2 Bass kernel for nn_Architecture_50629074485965 (3-layer AKT-style
transformer, B=16 S=512 D=1024 H=8 DFF=4096).

Sharding: data-parallel over batch — 2 batches per core, 8 cores, no
collectives.  Activations are feature-major [D on partitions, tokens free] so
every matmul chains without activation transposes (weights host-pre-
transposed).  Score path (K, q@k) runs in float32r; the value path (V, att,
probs, FFN) runs bf16.  Layer outputs bounce through DRAM.

All tile pools are persistent: tags rotate across batches and layers instead
of pool release/realloc, so the scheduler can overlap batch b1's projections
and attention with batch b0's FFN (PE-heavy vs ACT/DVE-heavy phases).

Every ACT transcendental is Exp or Ln (sqrt(x) = exp(0.5 ln x)) so a single
activation table set serves the whole kernel (no ~2.7us table swaps).

Attention per (b,h), per 128-row q-tile (q-major [q, k] layout):
  psum  = q @ k^T                         (PE f32r)
  e1    = Exp(psum/sqrt(dk))              (ACT, full width)
  e1c   = causal(e1)                      (GPSIMD affine_select, width w)
  r1    = sum_j e1*dam01                  (DVE stt accum -> throwaway edam;
                                           dam01 = u8 row-window gather from a
                                           per-head Toeplitz vector)
  cum   = cumsum(e1c)                     (DVE tensor_tensor_scan)
  d2    = (cum - rowtot) * (-|i-j|) >= 0  (DVE stt, posn = -|i-j| in f16)
  te    = exp(-exp(0.5 ln d2 + lgam - 0.5 ln r1))   (ACT Ln/Exp/Exp)
  t2u   = max(te,1e-5) * psum             (DVE stt; diag block causal-masked
                                           in place by GPSIMD)
  e2,r2 = Exp(t2u/sqrt(dk)) + row-sum     (ACT accum_out)
  probs = e2 * (1/max(r2,1e-30)) -> bf16  (DVE)
  probsT blocks: PE transpose -> psum -> sbuf
  att   = v-chunks(lhsT) @ probsT -> feature-major  (PE, bf16)
"""
import sys
sys.path.insert(0, "/opt/trn_rl_repo")
import numpy as np

B, S, D, H, DFF, LN_ = 16, 512, 1024, 8, 4096, 3
DK = D // H
NB = 2
TOK = NB * S
P = 128
ND = D // P      # 8
NQ = S // P      # 4
ISD = 1.0 / float(np.sqrt(DK))
WPAD = 2048

_CACHE = {}


def _build(nlayers=3, taps=(), repeat=1):
    import concourse.bass as bass
    import concourse.mybir as mybir
    from concourse import bacc
    from concourse.tile import TileContext

    dt = mybir.dt
    f32, f32r, bf16, f16, u8, i32 = (dt.float32, dt.float32r, dt.bfloat16,
                                     dt.float16, dt.uint8, dt.int32)
    AF = mybir.ActivationFunctionType
    OP = mybir.AluOpType

    nc = bacc.Bacc(None, target_bir_lowering=False)

    # Every transcendental in this kernel is Exp or Ln. The act-table-load
    # pass picks the first act_info set containing each function, which makes
    # Exp/Ln alternation swap tables every few ops (~2.7us per swap on HW).
    # Steer both to the combined natural_log_exp set by hiding them from the
    # single-function sets (dict identity is the functools.cache singleton;
    # set indices — what walrus consumes — are unchanged).
    from concourse.hw_specs import get_activation_tables
    _tabs = get_activation_tables(nc.m.arch)
    for _name, _fns in _tabs.items():
        if _name != "natural_log_exp_and_others":
            _fns.discard(AF.Exp)
            _fns.discard(AF.Ln)

    def par(name, shape, out=False, dtype=None):
        return nc.declare_dram_parameter(name, list(shape), dtype or f32,
                                         isOutput=out)

    xqa_e = par("xqa", [D, TOK], dtype=f32r)
    xq_e = par("xq", [D, TOK], dtype=f32r)
    kwt_e = par("kwt", [LN_, D, D], dtype=f32r)
    vwt_e = par("vwt", [LN_, D, D], dtype=f32r)
    owt_e = par("owt", [LN_, D, D], dtype=bf16)
    w1t_e = par("w1t", [LN_, D, DFF], dtype=bf16)
    w2t_e = par("w2t", [LN_, DFF, D], dtype=bf16)
    a0f_e = par("a0f", [LN_, H, S]); a1f_e = par("a1f", [LN_, H, S])
    e0f_e = par("e0f", [LN_, H, S]); e1f_e = par("e1f", [LN_, H, S])
    a0r_e = par("a0r", [LN_, H, S]); a1r_e = par("a1r", [LN_, H, S])
    e0r_e = par("e0r", [LN_, H, S]); e1r_e = par("e1r", [LN_, H, S])
    gam_e = par("gam", [1, LN_ * H])
    posn_e = par("posn", [S, S], dtype=f16)
    out_e = par("out", [D, TOK], out=True)
    tap_outs = {}

    with TileContext(nc) as tc:
        pg = tc.alloc_tile_pool(name="glob", bufs=1)
        pdram = tc.alloc_tile_pool(name="dram", bufs=1, space="DRAM")
        psQ = tc.alloc_tile_pool(name="psQ", bufs=3, space="PSUM")
        psT = tc.alloc_tile_pool(name="psT", bufs=1, space="PSUM")
        psAv = tc.alloc_tile_pool(name="psAv", bufs=1, space="PSUM")
        pool = tc.alloc_tile_pool(name="main", bufs=2)

        _dmaq = [nc.sync, nc.sync]
        _dmac = [0]

        def wdma(out, in_):
            eng = _dmaq[_dmac[0] % len(_dmaq)]
            _dmac[0] += 1
            eng.dma_start(out=out, in_=in_)

        def mm_group(psum_ap, pairs):
            n = len(pairs)
            for i, (lt, rh) in enumerate(pairs):
                nc.tensor.matmul(psum_ap, lt, rh,
                                 start=(i == 0), stop=(i == n - 1))

        # ---------------- constants (global pool) ----------------
        ident_f = pg.tile([P, P], f32, name="t", tag="identf")
        nc.gpsimd.memset(ident_f[:], 0.0)
        nc.gpsimd.affine_select(
            out=ident_f[:], in_=ident_f[:], compare_op=OP.not_equal,
            fill=1.0, base=0, channel_multiplier=1, pattern=[[-1, P]])
        ident_bf = pg.tile([P, P], bf16, name="t", tag="identbf")
        nc.vector.tensor_copy(ident_bf[:], ident_f[:])

        ones_f = pg.tile([P, 1], f32, name="t", tag="onesf")
        nc.gpsimd.memset(ones_f[:], 1.0)
        ones_col = pg.tile([P, 1], f32r, name="t", tag="ones")
        nc.vector.tensor_copy(ones_col[:], ones_f[:])
        eps5 = pg.tile([P, 1], f32, name="t", tag="eps5")
        nc.gpsimd.memset(eps5[:], 1e-5)

        posn = []
        for qt in range(NQ):
            t = pg.tile([P, S], f16, name="t", tag=f"posn{qt}")
            nc.sync.dma_start(out=t[:], in_=posn_e[qt * P:(qt + 1) * P, :])
            posn.append(t)

        idxt = []
        for h in range(H):
            t = pg.tile([P, 1], i32, name="t", tag=f"idx{h}")
            nc.gpsimd.iota(t[:], pattern=[[1, 1]],
                           base=h * WPAD + (S - 1) - P * (NQ - 1),
                           channel_multiplier=-1)
            idxt.append(t)

        grow = pg.tile([1, LN_ * H], f32, name="t", tag="grow")
        nc.sync.dma_start(out=grow[:], in_=gam_e[:])
        one_c = pg.tile([P, 1], f32, name="t", tag="one_c")
        nc.gpsimd.memset(one_c[:], 1.0)
        # softplus(x) = ln(1 + exp(x)) computed manually (no Softplus table)
        gsp = pg.tile([1, LN_ * H], f32, name="t", tag="gsp")
        nc.scalar.activation(gsp[:], grow[:], AF.Exp)
        nc.scalar.activation(gsp[:], gsp[:], AF.Ln, bias=one_c[:1, :])
        # lgam = ln(softplus(gamma)); te = exp(-exp(0.5*ln(d2)-0.5*ln(r1)+lgam))
        lgam = pg.tile([1, LN_ * H], f32, name="t", tag="lgam")
        nc.scalar.activation(lgam[:], gsp[:], AF.Ln)
        lgam_bc = []
        for i in range(LN_ * H):
            t = pg.tile([P, 1], f32, name="t", tag=f"gbc{i}")
            nc.gpsimd.partition_broadcast(t[:], lgam[0:1, i:i + 1])
            lgam_bc.append(t)

        y_dram = pdram.tile([D, TOK], f32r, name="t", tag="ydram")
        x1_dram = pdram.tile([D, TOK], f32r, name="t", tag="x1dram")

        # ---------------- helpers ----------------
        def dam_prep(l):
            wdam = pdram.tile([1, H * WPAD], u8, name="t", tag="wdam",
                              bufs=2)

            def half(a0e, a1e, e0e, e1e):
                tA = pool.tile([H, S], f32, name="t", tag="tmpA", bufs=4)
                tB = pool.tile([H, S], f32, name="t", tag="tmpB", bufs=3)
                tC = pool.tile([H, S], f32, name="t", tag="tmpA", bufs=4)
                tD = pool.tile([H, S], f32, name="t", tag="tmpB", bufs=3)
                nc.sync.dma_start(out=tA[:], in_=e0e[l])
                nc.sync.dma_start(out=tB[:], in_=e1e[l])
                nc.scalar.activation(tA[:], tA[:], AF.Ln, bias=eps5[:H, :])
                nc.scalar.activation(tB[:], tB[:], AF.Ln, bias=eps5[:H, :])
                nc.vector.tensor_tensor(tA[:], tA[:], tB[:], OP.subtract)
                nc.sync.dma_start(out=tC[:], in_=a1e[l])
                nc.sync.dma_start(out=tD[:], in_=a0e[l])
                nc.vector.tensor_tensor(tC[:], tC[:], tD[:], OP.subtract)
                nc.vector.tensor_tensor(tA[:], tA[:], tC[:], OP.add)
                c = pool.tile([H, S], u8, name="t", tag="edam", bufs=2)
                nc.vector.tensor_scalar(c[:], tA[:], 0.0, None, OP.is_gt)
                return c

            cf = half(a0f_e, a1f_e, e0f_e, e1f_e)
            cr = half(a0r_e, a1r_e, e0r_e, e1r_e)
            dst_r = bass.AP(tensor=wdam.tensor, offset=0,
                            ap=[[WPAD, H], [1, S - 1]])
            dst_f = bass.AP(tensor=wdam.tensor, offset=S - 1,
                            ap=[[WPAD, H], [1, S]])
            nc.sync.dma_start(out=dst_r, in_=cr[:, 0:S - 1])
            nc.sync.dma_start(out=dst_f, in_=cf[:])
            return wdam

        def layernorm(r_t, dsts):
            """r_t: 8 [P,S] f32r tiles; writes (x-mu)/sigma into dsts APs."""
            s1 = psT.tile([1, S], f32, name="t", tag="pt0")
            mm_group(s1[:], [(ones_col[:], r_t[od][:]) for od in range(ND)])
            s2 = psT.tile([1, S], f32, name="t", tag="pt1")
            for od in range(ND):
                sq = pool.tile([P, S], f32r, name="t", tag="tmpA", bufs=4)
                nc.vector.tensor_tensor(sq[:], r_t[od][:], r_t[od][:],
                                        OP.mult)
                nc.tensor.matmul(s2[:], ones_col[:], sq[:],
                                 start=(od == 0), stop=(od == ND - 1))
            mean = pool.tile([1, S], f32, name="t", tag="lnr0", bufs=2)
            nc.vector.tensor_scalar(mean[:], s1[:], 1.0 / D, None, OP.mult)
            msq = pool.tile([1, S], f32, name="t", tag="lnr1", bufs=2)
            nc.vector.tensor_scalar(msq[:], s2[:], 1.0 / D, None, OP.mult)
            m2 = pool.tile([1, S], f32, name="t", tag="lnr2", bufs=2)
            nc.vector.tensor_tensor(m2[:], mean[:], mean[:], OP.mult)
            nc.vector.tensor_tensor(msq[:], msq[:], m2[:], OP.subtract)
            # rstd = exp(-0.5*ln(var+eps)) — stays in the ln/exp table set
            nc.scalar.activation(msq[:], msq[:], AF.Ln, bias=eps5[:1, :])
            nc.scalar.activation(m2[:], msq[:], AF.Exp, scale=-0.5)
            nc.vector.tensor_scalar(mean[:], mean[:], -1.0, None, OP.mult)
            nc.vector.tensor_tensor(mean[:], mean[:], m2[:], OP.mult)
            Ab = pool.tile([P, S], f32, name="t", tag="Ab", bufs=1)
            nc.gpsimd.partition_broadcast(Ab[:], m2[:])
            Cb = pool.tile([P, S], f32, name="t", tag="Cb", bufs=1)
            nc.gpsimd.partition_broadcast(Cb[:], mean[:])
            for od in range(ND):
                t1 = pool.tile([P, S], f32, name="t", tag="tmpA", bufs=4)
                nc.vector.tensor_tensor(t1[:], r_t[od][:], Ab[:], OP.mult)
                nc.vector.tensor_tensor(dsts[od], t1[:], Cb[:], OP.add)

        def attention_head(l, bmask, h, K, V, att_dst, damG):
            pst = [psT.tile([P, S], bf16, name="t", tag=f"pt{kc}")
                   for kc in range(NQ)]
            ktile = K[h]
            for qt in range(NQ):
                w = P * (qt + 1)
                ps = psQ.tile([P, S], f32, name="t", tag="qk")
                mm_group(ps[:], [(ktile[:, qt * P:qt * P + P], ktile[:])])
                doff = P * (NQ - 1) - P * qt
                e1 = pool.tile([P, S], bf16, name="t", tag="e1", bufs=3)
                nc.scalar.activation(e1[:], ps[:], AF.Exp, scale=ISD)
                e1c = pool.tile([P, S], bf16, name="t", tag="e1c", bufs=2)
                nc.gpsimd.affine_select(
                    out=e1c[:, :w], in_=e1[:, :w], compare_op=OP.is_gt,
                    fill=0.0, base=qt * P + bmask, channel_multiplier=1,
                    pattern=[[-1, w]])
                r1 = pool.tile([P, 1], f32, name="t", tag="sm_r1")
                edam = pool.tile([P, S], bf16, name="t", tag="edam", bufs=2)
                nc.vector.scalar_tensor_tensor(
                    edam[:], e1[:], 1.0, damG[:, doff:doff + S],
                    OP.mult, OP.mult, accum_out=r1[:])
                cum = pool.tile([P, S], f32, name="t", tag="tmpB", bufs=3)
                nc.vector.tensor_tensor_scan(
                    cum[:, :w], e1c[:, :w], e1c[:, :w], 0.0, OP.add, OP.bypass)
                lnr1 = pool.tile([P, 1], f32, name="t", tag="sm_rc1")
                nc.scalar.activation(lnr1[:], r1[:], AF.Ln)
                brow = pool.tile([P, 1], f32, name="t", tag="sm_brow")
                nc.vector.scalar_tensor_tensor(
                    brow[:], lnr1[:], -0.5, lgam_bc[l * H + h][:],
                    OP.mult, OP.add)
                d2 = pool.tile([P, S], f32, name="t", tag="tmpA", bufs=4)
                nc.vector.scalar_tensor_tensor(
                    d2[:, :w], cum[:, :w], cum[:, w - 1:w], posn[qt][:, :w],
                    OP.subtract, OP.mult)
                dist = pool.tile([P, S], f32, name="t", tag="tmpB", bufs=3)
                nc.scalar.activation(dist[:, :w], d2[:, :w], AF.Ln)
                sga = pool.tile([P, S], f32, name="t", tag="tmpA", bufs=4)
                nc.scalar.activation(sga[:, :w], dist[:, :w], AF.Exp,
                                     scale=0.5, bias=brow[:])
                te = pool.tile([P, S], f32, name="t", tag="tmpB", bufs=3)
                nc.scalar.activation(te[:, :w], sga[:, :w], AF.Exp,
                                     scale=-1.0)
                t2u = pool.tile([P, S], f32, name="t", tag="tmpA", bufs=4)
                nc.vector.scalar_tensor_tensor(
                    t2u[:, :w], te[:, :w], 1e-5, ps[:, :w], OP.max, OP.mult)
                # causal boundary only cuts the 128-wide diagonal block;
                # mask it in place instead of re-writing the full width.
                nc.gpsimd.affine_select(
                    out=t2u[:, w - P:w], in_=t2u[:, w - P:w],
                    compare_op=OP.is_gt, fill=-1e30, base=bmask,
                    channel_multiplier=1, pattern=[[-1, P]])
                e2 = pool.tile([P, S], bf16, name="t", tag="tmpB", bufs=3)
                r2 = pool.tile([P, 1], f32, name="t", tag="sm_r2")
                nc.scalar.activation(e2[:, :w], t2u[:, :w], AF.Exp,
                                     scale=ISD, accum_out=r2[:])
                nc.vector.tensor_scalar(r2[:], r2[:], 1e-30, None, OP.max)
                rec2 = pool.tile([P, 1], f32, name="t", tag="sm_rc2")
                nc.vector.reciprocal(rec2[:], r2[:])
                pr = pool.tile([P, S], bf16, name="t", tag="probs", bufs=2)
                nc.vector.tensor_scalar(pr[:, :w], e2[:, :w], rec2[:],
                                        None, OP.mult)
                for kc in range(qt + 1):
                    nc.tensor.transpose(
                        pst[kc][:, qt * P:qt * P + P],
                        pr[:, kc * P:kc * P + P], ident_bf[:])
            prT = []
            for kc in range(NQ):
                t = pool.tile([P, S], bf16, name="t", tag=f"prT{kc}", bufs=1)
                nc.vector.tensor_copy(t[:, kc * P:], pst[kc][:, kc * P:])
                prT.append(t)
            pav = psAv.tile([P, S], f32, name="t", tag="av")
            for kc in range(NQ):
                nc.tensor.matmul(
                    pav[:, kc * P:], V[kc][:, h * DK:(h + 1) * DK],
                    prT[kc][:, kc * P:],
                    start=(kc == 0), stop=(kc == NQ - 1))
            nc.scalar.copy(att_dst, pav[:])

        def layer(l, bmask, apply_pos, xsrc_dram, vals_src, out_dram,
                  final=False):
            """xsrc_dram: [D, TOK] DRAM source for the query/key input.
            vals_src: 'self' or a DRAM tile to stream per b.
            out_dram: DRAM target AP base for the layer output."""
            wdam = dam_prep(l)
            damGs = []
            for h in range(H):
                g = pool.tile([P, 2 * S - P], u8, name="t", tag=f"damG{h}",
                              bufs=1)
                nc.gpsimd.indirect_dma_start(
                    out=g[:], out_offset=None, in_=wdam[:],
                    in_offset=bass.IndirectOffsetOnAxis(
                        ap=idxt[h][:, :1], axis=1))
                damGs.append(g)

            def proj(b):
                bs = b * S
                xq_tiles = []
                for idt in range(ND):
                    t = pool.tile([P, S], f32r, name="t", tag=f"xa{idt}",
                                  bufs=1)
                    wdma(t[:],
                         xsrc_dram[idt * P:(idt + 1) * P, bs:bs + S])
                    xq_tiles.append(t)
                K = []

                def khalf(half):
                    wk = []
                    for idt in range(ND):
                        t = pool.tile([P, S], f32r, name="t",
                                      tag=f"kw{idt}", bufs=1)
                        wdma(
                            t[:],
                            kwt_e[l, idt * P:(idt + 1) * P,
                                      half * S:(half + 1) * S])
                        wk.append(t)
                    for oc in range(4):
                        od = half * 4 + oc
                        ps = psQ.tile([P, S], f32, name="t", tag="qk")
                        mm_group(ps[:], [
                            (wk[idt][:, oc * P:(oc + 1) * P],
                             xq_tiles[idt][:]) for idt in range(ND)])
                        kt = pool.tile([P, S], bf16, name="t", tag=f"K{od}",
                                       bufs=1)
                        nc.vector.tensor_copy(kt[:], ps[:])
                        K.append(kt)

                khalf(0)
                if vals_src == "self":
                    vals = [xq_tiles[idt][:] for idt in range(ND)]
                else:
                    vt = []
                    for idt in range(ND):
                        t = pool.tile([P, S], f32r, name="t", tag=f"r{idt}",
                                      bufs=1)
                        wdma(
                            t[:],
                            vals_src[idt * P:(idt + 1) * P, bs:bs + S])
                        vt.append(t)
                    vals = [t[:] for t in vt]
                V = [pool.tile([P, D], bf16, name="t", tag=f"V{st}", bufs=1)
                     for st in range(NQ)]

                def vhalf(half):
                    wv = []
                    for idt in range(ND):
                        t = pool.tile([P, S], f32r, name="t",
                                      tag=f"kw{idt}", bufs=1)
                        wdma(
                            t[:],
                            vwt_e[l, idt * P:(idt + 1) * P,
                                      half * S:(half + 1) * S])
                        wv.append(t)
                    for st in range(NQ):
                        ps = psQ.tile([P, S], f32, name="t", tag="qk")
                        mm_group(ps[:], [
                            (vals[idt][:, st * P:(st + 1) * P], wv[idt][:])
                            for idt in range(ND)])
                        nc.vector.tensor_copy(
                            V[st][:, half * S:(half + 1) * S], ps[:])

                # The rest of the projection is emitted as prelude closures
                # popped inside the first attention heads, so the chains'
                # ACT/DVE work front-runs the projection matmuls on PE.
                vhalf(0)
                prelude = [lambda: khalf(1), lambda: vhalf(1)]
                return xq_tiles, K, V, prelude

            def att_phase(b, K, V, prelude):
                att = [pool.tile([P, S], bf16, name="t", tag=f"att{od}",
                                 bufs=5)
                       for od in range(ND)]
                for h in range(H):
                    attention_head(l, bmask, h, K, V, att[h][:], damGs[h])
                    if prelude:
                        prelude.pop(0)()
                    elif pend:
                        pend.pop(0)()
                while pend:
                    pend.pop(0)()
                return att

            def oln(b, xq_tiles, att):
                bs = b * S
                r_t = []
                for half in range(2):
                    wo = []
                    for idt in range(ND):
                        t = pool.tile([P, S], bf16, name="t",
                                      tag=f"wbig{idt}", bufs=2)
                        wdma(
                            t[:],
                            owt_e[l, idt * P:(idt + 1) * P,
                                      half * S:(half + 1) * S])
                        wo.append(t)
                    for oc in range(4):
                        od = half * 4 + oc
                        ps = psQ.tile([P, S], f32, name="t", tag="qk")
                        mm_group(ps[:], [
                            (wo[idt][:, oc * P:(oc + 1) * P], att[idt][:])
                            for idt in range(ND)])
                        rt = pool.tile([P, S], f32r, name="t",
                                       tag=f"r{od}", bufs=1)
                        nc.vector.tensor_tensor(
                            rt[:], xq_tiles[od][:], ps[:], OP.add)
                        r_t.append(rt)
                if apply_pos:
                    xp = [pg.tile([P, S], f32r, name="t", tag=f"xp{od}")
                          for od in range(ND)]
                    layernorm(r_t, [t[:] for t in xp])
                    return xp
                ot = [pool.tile([P, S], f32 if final else f32r, name="t",
                                tag="outt", bufs=2)
                      for _ in range(ND)]
                layernorm(r_t, [t[:] for t in ot])
                for od in range(ND):
                    nc.sync.dma_start(
                        out=out_dram[od * P:(od + 1) * P, bs:bs + S],
                        in_=ot[od][:])
                return None

            def make_ffn(b, xp):
                bs = b * S
                xpb = []
                for od in range(ND):
                    t = pool.tile([P, S], bf16, name="t", tag=f"xpb{od}",
                                  bufs=1)
                    nc.scalar.copy(t[:], xp[od][:])
                    xpb.append(t)
                h1 = []

                def w1_block(fc):
                    w1c = []
                    for idt in range(ND):
                        t = pool.tile([P, S], bf16, name="t",
                                      tag=f"wbig{idt}", bufs=2)
                        wdma(
                            t[:],
                            w1t_e[l, idt * P:(idt + 1) * P,
                                      fc * S:(fc + 1) * S])
                        w1c.append(t)
                    for fl in range(4):
                        ft = fc * 4 + fl
                        ps = psQ.tile([P, S], f32, name="t", tag="qk")
                        mm_group(ps[:], [
                            (w1c[idt][:, fl * P:(fl + 1) * P], xpb[idt][:])
                            for idt in range(ND)])
                        ht = pool.tile([P, S], bf16, name="t",
                                       tag=f"att{ft % 8}", bufs=5)
                        nc.vector.tensor_scalar(ht[:], ps[:], 0.0, None,
                                                OP.max)
                        h1.append(ht)

                def tail():
                    r_t = []
                    for og in range(2):
                        pso = [psT.tile([P, S], f32, name="t", tag=f"pt{oc}")
                               for oc in range(4)]
                        for fc in range(8):
                            w2c = []
                            for fl in range(4):
                                ft = fc * 4 + fl
                                t = pool.tile([P, S], bf16, name="t",
                                              tag=f"wbig{4 + fl}", bufs=2)
                                wdma(
                                    t[:],
                                    w2t_e[l, ft * P:(ft + 1) * P,
                                              og * S:(og + 1) * S])
                                w2c.append(t)
                            for fl in range(4):
                                ft = fc * 4 + fl
                                for oc in range(4):
                                    nc.tensor.matmul(
                                        pso[oc][:],
                                        w2c[fl][:, oc * P:(oc + 1) * P],
                                        h1[ft][:],
                                        start=(fc == 0 and fl == 0),
                                        stop=(fc == 7 and fl == 3))
                        for oc in range(4):
                            od = og * 4 + oc
                            rt = pool.tile([P, S], f32r, name="t",
                                           tag=f"r{od}", bufs=1)
                            nc.vector.tensor_tensor(
                                rt[:], xp[od][:], pso[oc][:], OP.add)
                            r_t.append(rt)
                    ot = [pool.tile([P, S], f32 if final else f32r, name="t",
                                    tag="outt", bufs=2)
                          for _ in range(ND)]
                    layernorm(r_t, [t[:] for t in ot])
                    for od in range(ND):
                        nc.sync.dma_start(
                            out=out_dram[od * P:(od + 1) * P, bs:bs + S],
                            in_=ot[od][:])

                return [lambda fc=fc: w1_block(fc) for fc in range(8)] + [tail]

            for b in range(NB):
                xq_tiles, K, V, prelude = proj(b)
                att = att_phase(b, K, V, prelude)
                xp = oln(b, xq_tiles, att)
                if apply_pos:
                    pend.extend(make_ffn(b, xp))

        # ================= driver =================
        pend = []
        for _rep in range(repeat):
            layer(0, 1, True, xqa_e, "self", y_dram)
            if nlayers >= 2:
                layer(1, 1, False, xq_e, "self", x1_dram)
            if nlayers >= 3:
                layer(2, 0, True, x1_dram, y_dram, out_e, final=True)
            while pend:
                pend.pop(0)()
            if nlayers == 1:
                nc.gpsimd.dma_start(out=out_e[:], in_=y_dram[:])
            elif nlayers == 2:
                nc.gpsimd.dma_start(out=out_e[:], in_=x1_dram[:])

        pool.release()
        psAv.release()
        psT.release()
        psQ.release()
        pdram.release()
        pg.release()

    nc.finalize()
    return nc, tap_outs


def _get_nc(nlayers=3, taps=(), repeat=1):
    key = (nlayers, tuple(sorted(taps)), repeat)
    if key not in _CACHE:
        _CACHE[key] = _build(nlayers, taps, repeat)
    return _CACHE[key]


def _make_in_maps(inputs):
    qa = np.asarray(inputs["qa_embed_data"])
    qd = np.asarray(inputs["q_embed_data"])
    al = np.asarray(inputs["alphas"])
    ge = np.asarray(inputs["gumbel_E"])
    a0f = al[..., 0]; a1f = al[..., 1]
    e0f = ge[..., 0]; e1f = ge[..., 1]
    i_ = np.arange(S)
    shared = {
        "kwt": np.asarray(inputs["kW"]).transpose(0, 2, 1),
        "vwt": np.asarray(inputs["vW"]).transpose(0, 2, 1),
        "owt": np.asarray(inputs["oW"]).transpose(0, 2, 1),
        "w1t": np.asarray(inputs["w1"]).transpose(0, 2, 1),
        "w2t": np.asarray(inputs["w2"]).transpose(0, 2, 1),
        "a0f": a0f, "a1f": a1f, "e0f": e0f, "e1f": e1f,
        "a0r": a0f[:, :, ::-1], "a1r": a1f[:, :, ::-1],
        "e0r": e0f[:, :, ::-1], "e1r": e1f[:, :, ::-1],
        "gam": np.asarray(inputs["gammas"]).reshape(1, LN_ * H),
        "posn": -np.abs(i_[:, None] - i_[None, :]),
    }
    import ml_dtypes
    casts = {"w1t": ml_dtypes.bfloat16, "w2t": ml_dtypes.bfloat16,
             "owt": ml_dtypes.bfloat16, "posn": np.float16}
    shared = {k: np.ascontiguousarray(v, dtype=casts.get(k, np.float32))
              for k, v in shared.items()}

    def feat_major(x, c):
        pair = np.asarray(x[NB * c:NB * c + NB])        # [2, S, D]
        return np.ascontiguousarray(
            pair.transpose(2, 0, 1).reshape(D, TOK), dtype=np.float32)

    in_maps = []
    for c in range(8):
        m = dict(shared)
        m["xqa"] = feat_major(qa, c)
        m["xq"] = feat_major(qd, c)
        in_maps.append(m)
    return in_maps


def _gather_out(results):
    outs = []
    for r in results:
        o = r["out"].reshape(D, NB, S).transpose(1, 2, 0)
        outs.append(o)
    return np.ascontiguousarray(np.concatenate(outs, axis=0))


def kernel(**inputs):
    from concourse.bass_utils import run_bass_kernel_spmd
    nc, _ = _get_nc()
    in_maps = _make_in_maps(inputs)
    res = run_bass_kernel_spmd(nc, in_maps, core_ids=list(range(8)))
    return _gather_out(res.results)


# revision 128
# speedup vs baseline: 1.0031x; 1.0031x over previous
"""Trainium2 Bass kernel for nn_Architecture_50629074485965 (3-layer AKT-style
transformer, B=16 S=512 D=1024 H=8 DFF=4096).

Sharding: data-parallel over batch — 2 batches per core, 8 cores, no
collectives.  Activations are feature-major [D on partitions, tokens free] so
every matmul chains without activation transposes (weights host-pre-
transposed).  Score path (K, q@k) runs in float32r; the value path (V, att,
probs, FFN) runs bf16.  Layer outputs bounce through DRAM.

All tile pools are persistent: tags rotate across batches and layers instead
of pool release/realloc, so the scheduler can overlap batch b1's projections
and attention with batch b0's FFN (PE-heavy vs ACT/DVE-heavy phases).

Every ACT transcendental is Exp or Ln (sqrt(x) = exp(0.5 ln x)) so a single
activation table set serves the whole kernel (no ~2.7us table swaps).

Attention per (b,h), per 128-row q-tile (q-major [q, k] layout):
  psum  = q @ k^T                         (PE f32r)
  e1    = Exp(psum/sqrt(dk))              (ACT, full width)
  e1c   = causal(e1)                      (GPSIMD affine_select, width w)
  r1    = sum_j e1*dam01                  (DVE stt accum -> throwaway edam;
                                           dam01 = u8 row-window gather from a
                                           per-head Toeplitz vector)
  cum   = cumsum(e1c)                     (DVE tensor_tensor_scan)
  d2    = (cum - rowtot) * (-|i-j|) >= 0  (DVE stt, posn = -|i-j| in f16)
  te    = exp(-exp(0.5 ln d2 + lgam - 0.5 ln r1))   (ACT Ln/Exp/Exp)
  t2u   = max(te,1e-5) * psum             (DVE stt; diag block causal-masked
                                           in place by GPSIMD)
  e2,r2 = Exp(t2u/sqrt(dk)) + row-sum     (ACT accum_out)
  probs = e2 * (1/max(r2,1e-30)) -> bf16  (DVE)
  probsT blocks: PE transpose -> psum -> sbuf
  att   = v-chunks(lhsT) @ probsT -> feature-major  (PE, bf16)
"""
import sys
sys.path.insert(0, "/opt/trn_rl_repo")
import numpy as np

B, S, D, H, DFF, LN_ = 16, 512, 1024, 8, 4096, 3
DK = D // H
NB = 2
TOK = NB * S
P = 128
ND = D // P      # 8
NQ = S // P      # 4
ISD = 1.0 / float(np.sqrt(DK))
WPAD = 2048

_CACHE = {}


def _build(nlayers=3, taps=(), repeat=1):
    import concourse.bass as bass
    import concourse.mybir as mybir
    from concourse import bacc
    from concourse.tile import TileContext

    dt = mybir.dt
    f32, f32r, bf16, f16, u8, i32 = (dt.float32, dt.float32r, dt.bfloat16,
                                     dt.float16, dt.uint8, dt.int32)
    AF = mybir.ActivationFunctionType
    OP = mybir.AluOpType

    nc = bacc.Bacc(None, target_bir_lowering=False)

    # Every transcendental in this kernel is Exp or Ln. The act-table-load
    # pass picks the first act_info set containing each function, which makes
    # Exp/Ln alternation swap tables every few ops (~2.7us per swap on HW).
    # Steer both to the combined natural_log_exp set by hiding them from the
    # single-function sets (dict identity is the functools.cache singleton;
    # set indices — what walrus consumes — are unchanged).
    from concourse.hw_specs import get_activation_tables
    _tabs = get_activation_tables(nc.m.arch)
    for _name, _fns in _tabs.items():
        if _name != "natural_log_exp_and_others":
            _fns.discard(AF.Exp)
            _fns.discard(AF.Ln)

    def par(name, shape, out=False, dtype=None):
        return nc.declare_dram_parameter(name, list(shape), dtype or f32,
                                         isOutput=out)

    xqa_e = par("xqa", [D, TOK], dtype=f32r)
    xq_e = par("xq", [D, TOK], dtype=f32r)
    kwt_e = par("kwt", [LN_, D, D], dtype=f32r)
    vwt_e = par("vwt", [LN_, D, D], dtype=f32r)
    owt_e = par("owt", [LN_, D, D], dtype=bf16)
    w1t_e = par("w1t", [LN_, D, DFF], dtype=bf16)
    w2t_e = par("w2t", [LN_, DFF, D], dtype=bf16)
    a0f_e = par("a0f", [LN_, H, S]); a1f_e = par("a1f", [LN_, H, S])
    e0f_e = par("e0f", [LN_, H, S]); e1f_e = par("e1f", [LN_, H, S])
    a0r_e = par("a0r", [LN_, H, S]); a1r_e = par("a1r", [LN_, H, S])
    e0r_e = par("e0r", [LN_, H, S]); e1r_e = par("e1r", [LN_, H, S])
    gam_e = par("gam", [1, LN_ * H])
    posn_e = par("posn", [S, S], dtype=f16)
    out_e = par("out", [D, TOK], out=True)
    tap_outs = {}

    with TileContext(nc) as tc:
        pg = tc.alloc_tile_pool(name="glob", bufs=1)
        pdram = tc.alloc_tile_pool(name="dram", bufs=1, space="DRAM")
        psQ = tc.alloc_tile_pool(name="psQ", bufs=3, space="PSUM")
        psT = tc.alloc_tile_pool(name="psT", bufs=1, space="PSUM")
        psAv = tc.alloc_tile_pool(name="psAv", bufs=1, space="PSUM")
        pool = tc.alloc_tile_pool(name="main", bufs=2)

        _dmaq = [nc.sync, nc.sync]
        _dmac = [0]

        def wdma(out, in_):
            eng = _dmaq[_dmac[0] % len(_dmaq)]
            _dmac[0] += 1
            eng.dma_start(out=out, in_=in_)

        def mm_group(psum_ap, pairs):
            n = len(pairs)
            for i, (lt, rh) in enumerate(pairs):
                nc.tensor.matmul(psum_ap, lt, rh,
                                 start=(i == 0), stop=(i == n - 1))

        # ---------------- constants (global pool) ----------------
        ident_f = pg.tile([P, P], f32, name="t", tag="identf")
        nc.gpsimd.memset(ident_f[:], 0.0)
        nc.gpsimd.affine_select(
            out=ident_f[:], in_=ident_f[:], compare_op=OP.not_equal,
            fill=1.0, base=0, channel_multiplier=1, pattern=[[-1, P]])
        ident_bf = pg.tile([P, P], bf16, name="t", tag="identbf")
        nc.vector.tensor_copy(ident_bf[:], ident_f[:])

        ones_f = pg.tile([P, 1], f32, name="t", tag="onesf")
        nc.gpsimd.memset(ones_f[:], 1.0)
        ones_col = pg.tile([P, 1], f32r, name="t", tag="ones")
        nc.vector.tensor_copy(ones_col[:], ones_f[:])
        eps5 = pg.tile([P, 1], f32, name="t", tag="eps5")
        nc.gpsimd.memset(eps5[:], 1e-5)

        posn = []
        for qt in range(NQ):
            t = pg.tile([P, S], f16, name="t", tag=f"posn{qt}")
            nc.sync.dma_start(out=t[:], in_=posn_e[qt * P:(qt + 1) * P, :])
            posn.append(t)

        idxt = []
        for h in range(H):
            t = pg.tile([P, 1], i32, name="t", tag=f"idx{h}")
            nc.gpsimd.iota(t[:], pattern=[[1, 1]],
                           base=h * WPAD + (S - 1) - P * (NQ - 1),
                           channel_multiplier=-1)
            idxt.append(t)

        grow = pg.tile([1, LN_ * H], f32, name="t", tag="grow")
        nc.sync.dma_start(out=grow[:], in_=gam_e[:])
        one_c = pg.tile([P, 1], f32, name="t", tag="one_c")
        nc.gpsimd.memset(one_c[:], 1.0)
        # softplus(x) = ln(1 + exp(x)) computed manually (no Softplus table)
        gsp = pg.tile([1, LN_ * H], f32, name="t", tag="gsp")
        nc.scalar.activation(gsp[:], grow[:], AF.Exp)
        nc.scalar.activation(gsp[:], gsp[:], AF.Ln, bias=one_c[:1, :])
        # lgam = ln(softplus(gamma)); te = exp(-exp(0.5*ln(d2)-0.5*ln(r1)+lgam))
        lgam = pg.tile([1, LN_ * H], f32, name="t", tag="lgam")
        nc.scalar.activation(lgam[:], gsp[:], AF.Ln)
        lgam_bc = []
        for i in range(LN_ * H):
            t = pg.tile([P, 1], f32, name="t", tag=f"gbc{i}")
            nc.gpsimd.partition_broadcast(t[:], lgam[0:1, i:i + 1])
            lgam_bc.append(t)

        y_dram = pdram.tile([D, TOK], f32r, name="t", tag="ydram")
        x1_dram = pdram.tile([D, TOK], f32r, name="t", tag="x1dram")

        # ---------------- helpers ----------------
        def dam_prep(l):
            wdam = pdram.tile([1, H * WPAD], u8, name="t", tag="wdam",
                              bufs=2)

            def half(a0e, a1e, e0e, e1e):
                tA = pool.tile([H, S], f32, name="t", tag="tmpA", bufs=4)
                tB = pool.tile([H, S], f32, name="t", tag="tmpB", bufs=3)
                tC = pool.tile([H, S], f32, name="t", tag="tmpA", bufs=4)
                tD = pool.tile([H, S], f32, name="t", tag="tmpB", bufs=3)
                nc.sync.dma_start(out=tA[:], in_=e0e[l])
                nc.sync.dma_start(out=tB[:], in_=e1e[l])
                nc.scalar.activation(tA[:], tA[:], AF.Ln, bias=eps5[:H, :])
                nc.scalar.activation(tB[:], tB[:], AF.Ln, bias=eps5[:H, :])
                nc.vector.tensor_tensor(tA[:], tA[:], tB[:], OP.subtract)
                nc.sync.dma_start(out=tC[:], in_=a1e[l])
                nc.sync.dma_start(out=tD[:], in_=a0e[l])
                nc.vector.tensor_tensor(tC[:], tC[:], tD[:], OP.subtract)
                nc.vector.tensor_tensor(tA[:], tA[:], tC[:], OP.add)
                c = pool.tile([H, S], u8, name="t", tag="edam", bufs=2)
                nc.vector.tensor_scalar(c[:], tA[:], 0.0, None, OP.is_gt)
                return c

            cf = half(a0f_e, a1f_e, e0f_e, e1f_e)
            cr = half(a0r_e, a1r_e, e0r_e, e1r_e)
            dst_r = bass.AP(tensor=wdam.tensor, offset=0,
                            ap=[[WPAD, H], [1, S - 1]])
            dst_f = bass.AP(tensor=wdam.tensor, offset=S - 1,
                            ap=[[WPAD, H], [1, S]])
            nc.sync.dma_start(out=dst_r, in_=cr[:, 0:S - 1])
            nc.sync.dma_start(out=dst_f, in_=cf[:])
            return wdam

        def layernorm(r_t, dsts):
            """r_t: 8 [P,S] f32r tiles; writes (x-mu)/sigma into dsts APs."""
            s1 = psT.tile([1, S], f32, name="t", tag="pt0")
            mm_group(s1[:], [(ones_col[:], r_t[od][:]) for od in range(ND)])
            s2 = psT.tile([1, S], f32, name="t", tag="pt1")
            for od in range(ND):
                sq = pool.tile([P, S], f32r, name="t", tag="tmpA", bufs=4)
                nc.vector.tensor_tensor(sq[:], r_t[od][:], r_t[od][:],
                                        OP.mult)
                nc.tensor.matmul(s2[:], ones_col[:], sq[:],
                                 start=(od == 0), stop=(od == ND - 1))
            mean = pool.tile([1, S], f32, name="t", tag="lnr0", bufs=2)
            nc.scalar.activation(mean[:], s1[:], AF.Copy, scale=1.0 / D)
            msq = pool.tile([1, S], f32, name="t", tag="lnr1", bufs=2)
            nc.scalar.activation(msq[:], s2[:], AF.Copy, scale=1.0 / D)
            m2 = pool.tile([1, S], f32, name="t", tag="lnr2", bufs=2)
            nc.vector.tensor_tensor(m2[:], mean[:], mean[:], OP.mult)
            nc.vector.tensor_tensor(msq[:], msq[:], m2[:], OP.subtract)
            # rstd = exp(-0.5*ln(var+eps)) — stays in the ln/exp table set
            nc.scalar.activation(msq[:], msq[:], AF.Ln, bias=eps5[:1, :])
            nc.scalar.activation(m2[:], msq[:], AF.Exp, scale=-0.5)
            nc.vector.tensor_scalar(mean[:], mean[:], -1.0, None, OP.mult)
            nc.vector.tensor_tensor(mean[:], mean[:], m2[:], OP.mult)
            Ab = pool.tile([P, S], f32, name="t", tag="Ab", bufs=1)
            nc.gpsimd.partition_broadcast(Ab[:], m2[:])
            Cb = pool.tile([P, S], f32, name="t", tag="Cb", bufs=1)
            nc.gpsimd.partition_broadcast(Cb[:], mean[:])
            for od in range(ND):
                t1 = pool.tile([P, S], f32, name="t", tag="tmpA", bufs=4)
                nc.vector.tensor_tensor(t1[:], r_t[od][:], Ab[:], OP.mult)
                nc.vector.tensor_tensor(dsts[od], t1[:], Cb[:], OP.add)

        def attention_head(l, bmask, h, K, V, att_dst, damG):
            pst = [psT.tile([P, S], bf16, name="t", tag=f"pt{kc}")
                   for kc in range(NQ)]
            ktile = K[h]
            for qt in range(NQ):
                w = P * (qt + 1)
                ps = psQ.tile([P, S], f32, name="t", tag="qk")
                mm_group(ps[:], [(ktile[:, qt * P:qt * P + P], ktile[:])])
                doff = P * (NQ - 1) - P * qt
                e1 = pool.tile([P, S], bf16, name="t", tag="e1", bufs=3)
                nc.scalar.activation(e1[:], ps[:], AF.Exp, scale=ISD)
                e1c = pool.tile([P, S], bf16, name="t", tag="e1c", bufs=2)
                nc.gpsimd.affine_select(
                    out=e1c[:, :w], in_=e1[:, :w], compare_op=OP.is_gt,
                    fill=0.0, base=qt * P + bmask, channel_multiplier=1,
                    pattern=[[-1, w]])
                r1 = pool.tile([P, 1], f32, name="t", tag="sm_r1")
                edam = pool.tile([P, S], bf16, name="t", tag="edam", bufs=2)
                nc.vector.scalar_tensor_tensor(
                    edam[:], e1[:], 1.0, damG[:, doff:doff + S],
                    OP.mult, OP.mult, accum_out=r1[:])
                cum = pool.tile([P, S], f32, name="t", tag="tmpB", bufs=3)
                nc.vector.tensor_tensor_scan(
                    cum[:, :w], e1c[:, :w], e1c[:, :w], 0.0, OP.add, OP.bypass)
                lnr1 = pool.tile([P, 1], f32, name="t", tag="sm_rc1")
                nc.scalar.activation(lnr1[:], r1[:], AF.Ln)
                brow = pool.tile([P, 1], f32, name="t", tag="sm_brow")
                nc.vector.scalar_tensor_tensor(
                    brow[:], lnr1[:], -0.5, lgam_bc[l * H + h][:],
                    OP.mult, OP.add)
                d2 = pool.tile([P, S], f32, name="t", tag="tmpA", bufs=4)
                nc.vector.scalar_tensor_tensor(
                    d2[:, :w], cum[:, :w], cum[:, w - 1:w], posn[qt][:, :w],
                    OP.subtract, OP.mult)
                dist = pool.tile([P, S], f32, name="t", tag="tmpB", bufs=3)
                nc.scalar.activation(dist[:, :w], d2[:, :w], AF.Ln)
                sga = pool.tile([P, S], f32, name="t", tag="tmpA", bufs=4)
                nc.scalar.activation(sga[:, :w], dist[:, :w], AF.Exp,
                                     scale=0.5, bias=brow[:])
                te = pool.tile([P, S], f32, name="t", tag="tmpB", bufs=3)
                nc.scalar.activation(te[:, :w], sga[:, :w], AF.Exp,
                                     scale=-1.0)
                t2u = pool.tile([P, S], f32, name="t", tag="tmpA", bufs=4)
                nc.vector.scalar_tensor_tensor(
                    t2u[:, :w], te[:, :w], 1e-5, ps[:, :w], OP.max, OP.mult)
                # causal boundary only cuts the 128-wide diagonal block;
                # mask it in place instead of re-writing the full width.
                nc.gpsimd.affine_select(
                    out=t2u[:, w - P:w], in_=t2u[:, w - P:w],
                    compare_op=OP.is_gt, fill=-1e30, base=bmask,
                    channel_multiplier=1, pattern=[[-1, P]])
                e2 = pool.tile([P, S], bf16, name="t", tag="tmpB", bufs=3)
                r2 = pool.tile([P, 1], f32, name="t", tag="sm_r2")
                nc.scalar.activation(e2[:, :w], t2u[:, :w], AF.Exp,
                                     scale=ISD, accum_out=r2[:])
                nc.vector.tensor_scalar(r2[:], r2[:], 1e-30, None, OP.max)
                rec2 = pool.tile([P, 1], f32, name="t", tag="sm_rc2")
                nc.vector.reciprocal(rec2[:], r2[:])
                pr = pool.tile([P, S], bf16, name="t", tag="probs", bufs=2)
                nc.vector.tensor_scalar(pr[:, :w], e2[:, :w], rec2[:],
                                        None, OP.mult)
                for kc in range(qt + 1):
                    nc.tensor.transpose(
                        pst[kc][:, qt * P:qt * P + P],
                        pr[:, kc * P:kc * P + P], ident_bf[:])
            prT = []
            for kc in range(NQ):
                t = pool.tile([P, S], bf16, name="t", tag=f"prT{kc}", bufs=1)
                nc.vector.tensor_copy(t[:, kc * P:], pst[kc][:, kc * P:])
                prT.append(t)
            pav = psAv.tile([P, S], f32, name="t", tag="av")
            for kc in range(NQ):
                nc.tensor.matmul(
                    pav[:, kc * P:], V[kc][:, h * DK:(h + 1) * DK],
                    prT[kc][:, kc * P:],
                    start=(kc == 0), stop=(kc == NQ - 1))
            nc.scalar.copy(att_dst, pav[:])

        def layer(l, bmask, apply_pos, xsrc_dram, vals_src, out_dram,
                  final=False):
            """xsrc_dram: [D, TOK] DRAM source for the query/key input.
            vals_src: 'self' or a DRAM tile to stream per b.
            out_dram: DRAM target AP base for the layer output."""
            wdam = dam_prep(l)
            damGs = []
            for h in range(H):
                g = pool.tile([P, 2 * S - P], u8, name="t", tag=f"damG{h}",
                              bufs=1)
                nc.gpsimd.indirect_dma_start(
                    out=g[:], out_offset=None, in_=wdam[:],
                    in_offset=bass.IndirectOffsetOnAxis(
                        ap=idxt[h][:, :1], axis=1))
                damGs.append(g)

            def proj(b):
                bs = b * S
                xq_tiles = []
                for idt in range(ND):
                    t = pool.tile([P, S], f32r, name="t", tag=f"xa{idt}",
                                  bufs=1)
                    wdma(t[:],
                         xsrc_dram[idt * P:(idt + 1) * P, bs:bs + S])
                    xq_tiles.append(t)
                K = []

                def khalf(half):
                    wk = []
                    for idt in range(ND):
                        t = pool.tile([P, S], f32r, name="t",
                                      tag=f"kw{idt}", bufs=1)
                        wdma(
                            t[:],
                            kwt_e[l, idt * P:(idt + 1) * P,
                                      half * S:(half + 1) * S])
                        wk.append(t)
                    for oc in range(4):
                        od = half * 4 + oc
                        ps = psQ.tile([P, S], f32, name="t", tag="qk")
                        mm_group(ps[:], [
                            (wk[idt][:, oc * P:(oc + 1) * P],
                             xq_tiles[idt][:]) for idt in range(ND)])
                        kt = pool.tile([P, S], bf16, name="t", tag=f"K{od}",
                                       bufs=1)
                        nc.vector.tensor_copy(kt[:], ps[:])
                        K.append(kt)

                khalf(0)
                if vals_src == "self":
                    vals = [xq_tiles[idt][:] for idt in range(ND)]
                else:
                    vt = []
                    for idt in range(ND):
                        t = pool.tile([P, S], f32r, name="t", tag=f"r{idt}",
                                      bufs=1)
                        wdma(
                            t[:],
                            vals_src[idt * P:(idt + 1) * P, bs:bs + S])
                        vt.append(t)
                    vals = [t[:] for t in vt]
                V = [pool.tile([P, D], bf16, name="t", tag=f"V{st}", bufs=1)
                     for st in range(NQ)]

                def vhalf(half):
                    wv = []
                    for idt in range(ND):
                        t = pool.tile([P, S], f32r, name="t",
                                      tag=f"kw{idt}", bufs=1)
                        wdma(
                            t[:],
                            vwt_e[l, idt * P:(idt + 1) * P,
                                      half * S:(half + 1) * S])
                        wv.append(t)
                    for st in range(NQ):
                        ps = psQ.tile([P, S], f32, name="t", tag="qk")
                        mm_group(ps[:], [
                            (vals[idt][:, st * P:(st + 1) * P], wv[idt][:])
                            for idt in range(ND)])
                        nc.vector.tensor_copy(
                            V[st][:, half * S:(half + 1) * S], ps[:])

                # The rest of the projection is emitted as prelude closures
                # popped inside the first attention heads, so the chains'
                # ACT/DVE work front-runs the projection matmuls on PE.
                vhalf(0)
                prelude = [lambda: khalf(1), lambda: vhalf(1)]
                return xq_tiles, K, V, prelude

            def att_phase(b, K, V, prelude):
                att = [pool.tile([P, S], bf16, name="t", tag=f"att{od}",
                                 bufs=5)
                       for od in range(ND)]
                for h in range(H):
                    attention_head(l, bmask, h, K, V, att[h][:], damGs[h])
                    if prelude:
                        prelude.pop(0)()
                    elif pend:
                        pend.pop(0)()
                while pend:
                    pend.pop(0)()
                return att

            def oln(b, xq_tiles, att):
                bs = b * S
                r_t = []
                for half in range(2):
                    wo = []
                    for idt in range(ND):
                        t = pool.tile([P, S], bf16, name="t",
                                      tag=f"wbig{idt}", bufs=2)
                        wdma(
                            t[:],
                            owt_e[l, idt * P:(idt + 1) * P,
                                      half * S:(half + 1) * S])
                        wo.append(t)
                    for oc in range(4):
                        od = half * 4 + oc
                        ps = psQ.tile([P, S], f32, name="t", tag="qk")
                        mm_group(ps[:], [
                            (wo[idt][:, oc * P:(oc + 1) * P], att[idt][:])
                            for idt in range(ND)])
                        rt = pool.tile([P, S], f32r, name="t",
                                       tag=f"r{od}", bufs=1)
                        nc.vector.tensor_tensor(
                            rt[:], xq_tiles[od][:], ps[:], OP.add)
                        r_t.append(rt)
                if apply_pos:
                    xp = [pg.tile([P, S], f32r, name="t", tag=f"xp{od}")
                          for od in range(ND)]
                    layernorm(r_t, [t[:] for t in xp])
                    return xp
                ot = [pool.tile([P, S], f32 if final else f32r, name="t",
                                tag="outt", bufs=2)
                      for _ in range(ND)]
                layernorm(r_t, [t[:] for t in ot])
                for od in range(ND):
                    nc.sync.dma_start(
                        out=out_dram[od * P:(od + 1) * P, bs:bs + S],
                        in_=ot[od][:])
                return None

            def make_ffn(b, xp):
                bs = b * S
                xpb = []
                for od in range(ND):
                    t = pool.tile([P, S], bf16, name="t", tag=f"xpb{od}",
                                  bufs=1)
                    nc.scalar.copy(t[:], xp[od][:])
                    xpb.append(t)
                h1 = []

                def w1_block(fc):
                    w1c = []
                    for idt in range(ND):
                        t = pool.tile([P, S], bf16, name="t",
                                      tag=f"wbig{idt}", bufs=2)
                        wdma(
                            t[:],
                            w1t_e[l, idt * P:(idt + 1) * P,
                                      fc * S:(fc + 1) * S])
                        w1c.append(t)
                    for fl in range(4):
                        ft = fc * 4 + fl
                        ps = psQ.tile([P, S], f32, name="t", tag="qk")
                        mm_group(ps[:], [
                            (w1c[idt][:, fl * P:(fl + 1) * P], xpb[idt][:])
                            for idt in range(ND)])
                        ht = pool.tile([P, S], bf16, name="t",
                                       tag=f"att{ft % 8}", bufs=5)
                        nc.vector.tensor_scalar(ht[:], ps[:], 0.0, None,
                                                OP.max)
                        h1.append(ht)

                def tail():
                    r_t = []
                    for og in range(2):
                        pso = [psT.tile([P, S], f32, name="t", tag=f"pt{oc}")
                               for oc in range(4)]
                        for fc in range(8):
                            w2c = []
                            for fl in range(4):
                                ft = fc * 4 + fl
                                t = pool.tile([P, S], bf16, name="t",
                                              tag=f"wbig{4 + fl}", bufs=2)
                                wdma(
                                    t[:],
                                    w2t_e[l, ft * P:(ft + 1) * P,
                                              og * S:(og + 1) * S])
                                w2c.append(t)
                            for fl in range(4):
                                ft = fc * 4 + fl
                                for oc in range(4):
                                    nc.tensor.matmul(
                                        pso[oc][:],
                                        w2c[fl][:, oc * P:(oc + 1) * P],
                                        h1[ft][:],
                                        start=(fc == 0 and fl == 0),
                                        stop=(fc == 7 and fl == 3))
                        for oc in range(4):
                            od = og * 4 + oc
                            rt = pool.tile([P, S], f32r, name="t",
                                           tag=f"r{od}", bufs=1)
                            nc.vector.tensor_tensor(
                                rt[:], xp[od][:], pso[oc][:], OP.add)
                            r_t.append(rt)
                    ot = [pool.tile([P, S], f32 if final else f32r, name="t",
                                    tag="outt", bufs=2)
                          for _ in range(ND)]
                    layernorm(r_t, [t[:] for t in ot])
                    for od in range(ND):
                        nc.sync.dma_start(
                            out=out_dram[od * P:(od + 1) * P, bs:bs + S],
                            in_=ot[od][:])

                return [lambda fc=fc: w1_block(fc) for fc in range(8)] + [tail]

            for b in range(NB):
                xq_tiles, K, V, prelude = proj(b)
                att = att_phase(b, K, V, prelude)
                xp = oln(b, xq_tiles, att)
                if apply_pos:
                    pend.extend(make_ffn(b, xp))

        # ================= driver =================
        pend = []
        for _rep in range(repeat):
            layer(0, 1, True, xqa_e, "self", y_dram)
            if nlayers >= 2:
                layer(1, 1, False, xq_e, "self", x1_dram)
            if nlayers >= 3:
                layer(2, 0, True, x1_dram, y_dram, out_e, final=True)
            while pend:
                pend.pop(0)()
            if nlayers == 1:
                nc.gpsimd.dma_start(out=out_e[:], in_=y_dram[:])
            elif nlayers == 2:
                nc.gpsimd.dma_start(out=out_e[:], in_=x1_dram[:])

        pool.release()
        psAv.release()
        psT.release()
        psQ.release()
        pdram.release()
        pg.release()

    nc.finalize()
    return nc, tap_outs


def _get_nc(nlayers=3, taps=(), repeat=1):
    key = (nlayers, tuple(sorted(taps)), repeat)
    if key not in _CACHE:
        _CACHE[key] = _build(nlayers, taps, repeat)
    return _CACHE[key]


def _make_in_maps(inputs):
    qa = np.asarray(inputs["qa_embed_data"])
    qd = np.asarray(inputs["q_embed_data"])
    al = np.asarray(inputs["alphas"])
    ge = np.asarray(inputs["gumbel_E"])
    a0f = al[..., 0]; a1f = al[..., 1]
    e0f = ge[..., 0]; e1f = ge[..., 1]
    i_ = np.arange(S)
    shared = {
        "kwt": np.asarray(inputs["kW"]).transpose(0, 2, 1),
        "vwt": np.asarray(inputs["vW"]).transpose(0, 2, 1),
        "owt": np.asarray(inputs["oW"]).transpose(0, 2, 1),
        "w1t": np.asarray(inputs["w1"]).transpose(0, 2, 1),
        "w2t": np.asarray(inputs["w2"]).transpose(0, 2, 1),
        "a0f": a0f, "a1f": a1f, "e0f": e0f, "e1f": e1f,
        "a0r": a0f[:, :, ::-1], "a1r": a1f[:, :, ::-1],
        "e0r": e0f[:, :, ::-1], "e1r": e1f[:, :, ::-1],
        "gam": np.asarray(inputs["gammas"]).reshape(1, LN_ * H),
        "posn": -np.abs(i_[:, None] - i_[None, :]),
    }
    import ml_dtypes
    casts = {"w1t": ml_dtypes.bfloat16, "w2t": ml_dtypes.bfloat16,
             "owt": ml_dtypes.bfloat16, "posn": np.float16}
    shared = {k: np.ascontiguousarray(v, dtype=casts.get(k, np.float32))
              for k, v in shared.items()}

    def feat_major(x, c):
        pair = np.asarray(x[NB * c:NB * c + NB])        # [2, S, D]
        return np.ascontiguousarray(
            pair.transpose(2, 0, 1).reshape(D, TOK), dtype=np.float32)

    in_maps = []
    for c in range(8):
        m = dict(shared)
        m["xqa"] = feat_major(qa, c)
        m["xq"] = feat_major(qd, c)
        in_maps.append(m)
    return in_maps


def _gather_out(results):
    outs = []
    for r in results:
        o = r["out"].reshape(D, NB, S).transpose(1, 2, 0)
        outs.append(o)
    return np.ascontiguousarray(np.concatenate(outs, axis=0))


def kernel(**inputs):
    from concourse.bass_utils import run_bass_kernel_spmd
    nc, _ = _get_nc()
    in_maps = _make_in_maps(inputs)
    res = run_bass_kernel_spmd(nc, in_maps, core_ids=list(range(8)))
    return _gather_out(res.results)
